# revision 9
# baseline (speedup 1.0000x reference)
"""Quantized (4-bit) LoRA linear for Trainium2, SPMD over 8 NeuronCores.

Math:  y[t,o] = sum_i x[t,i]*W[o,i] + bias[o] + 2.0 * sum_r (x@A^T)[t,r]*B[o,r]
where  W[o,i] = (nib[o,i] - zero[i]) * scale[i],  nib = unpacked 4-bit ints.

Rewrite with xs[t,i] = x[t,i]*scale[i]:
  y[t,o] = sum_i xs[t,i]*nib[o,i]          (big fp16 matmul, nib exact in fp16)
         + G[t,:] @ H[o,:]^T               (K=10 matmul folding LoRA+zero+bias)
  G cols: 0-7 = u[t,r] = sum_i xs[t,i]*(A[r,i]/scale[i]),  8 = c[t] = sum_i xs*zero,
          9 = 1
  H rows: 0-7 = 2.0*B^T, 8 = -1, 9 = bias

Sharding: 8 cores = 4 token-quarters (2048 tok) x 2 out-halves (2048 outs).
Each core: nib tiles resident in SBUF, xs streamed as stationary [128,128]
slices, all accumulation in PSUM (6 banks main + 2 banks for u).
"""

import numpy as np

B, S, I, O = 4, 2048, 4096, 4096
T = B * S            # 8192 tokens
NQ, NH = 4, 2        # token quarters x out halves
TC = T // NQ         # 2048 tokens per core
OC = O // NH         # 2048 outs per core
KC = I // 128        # 32 contraction chunks
NTT = TC // 128      # 16 token tiles per core
TG = 4               # token tiles per token-group (512 tokens)
NG = NTT // TG       # 4 token groups

_CACHE = {}


def _build_program():
    import concourse.bacc as bacc
    import concourse.mybir as mybir
    import concourse.tile as tile

    fp16 = mybir.dt.float16
    fp32 = mybir.dt.float32
    fp8 = mybir.dt.float8e4

    nc = bacc.Bacc("TRN2", target_bir_lowering=False, debug=False)
    xsT = nc.dram_tensor("xsT", [I, TC], fp16, kind="ExternalInput")
    nibT = nc.dram_tensor("nibT", [I, OC], fp8, kind="ExternalInput")
    aextT = nc.dram_tensor("aextT", [I, 9], fp16, kind="ExternalInput")
    hmat = nc.dram_tensor("hmat", [9, OC], fp16, kind="ExternalInput")
    bias_bc = nc.dram_tensor("bias_bc", [128, OC], fp32, kind="ExternalInput")
    y = nc.dram_tensor("y", [TC, OC], fp32, kind="ExternalOutput")

    with tile.TileContext(nc) as tc:
        with (
            tc.tile_pool(name="nib", bufs=KC) as nib_pool,
            tc.tile_pool(name="consts", bufs=1) as const_pool,
            tc.tile_pool(name="xs", bufs=48) as xs_pool,
            tc.tile_pool(name="g", bufs=2) as g_pool,
            tc.tile_pool(name="out", bufs=3) as out_pool,
            tc.tile_pool(name="psum", bufs=7, space="PSUM") as psum_pool,
            tc.tile_pool(name="psum_u", bufs=1, space="PSUM") as psum_u_pool,
        ):
            nib_tiles = [None] * KC
            aext_tiles = [None] * KC
            h_tile = const_pool.tile([9, OC], fp16, tag="h")
            bias_tile = const_pool.tile([128, OC], fp32, tag="bias")

            for tg in range(NG):
                t0 = tg * TG * 128  # first token of group
                xs_tiles = []
                for k in range(KC):
                    # tg0: interleave weight/xs/aext DMAs per chunk so the PE
                    # can start after the first chunk lands instead of after
                    # the whole weight load.
                    if tg == 0:
                        nt = nib_pool.tile([128, OC], fp8, tag="nib",
                                           name=f"nib{k}")
                        nc.sync.dma_start(nt[:], nibT[k * 128:(k + 1) * 128, :])
                        nib_tiles[k] = nt
                    xt = xs_pool.tile([128, TG * 128], fp16, tag="xs",
                                      name=f"xs{tg}_{k}")
                    nc.sync.dma_start(
                        xt[:], xsT[k * 128:(k + 1) * 128, t0:t0 + TG * 128]
                    )
                    xs_tiles.append(xt)
                    if tg == 0:
                        at = const_pool.tile([128, 9], fp16, tag=f"aext{k}",
                                             name=f"aext{k}")
                        nc.sync.dma_start(at[:], aextT[k * 128:(k + 1) * 128, :])
                        aext_tiles[k] = at
                if tg == 0:
                    nc.sync.dma_start(h_tile[:], hmat[:, :])
                    nc.sync.dma_start(bias_tile[:], bias_bc[:, :])

                # u/c accumulation for this token group: psum [9, 512].
                # tg0: interleave tt0's main matmuls chunk-by-chunk with the
                # u matmuls so the PE streams right behind the DMA.
                up = psum_u_pool.tile([9, TG * 128], fp32, tag="u")
                ps0 = [
                    psum_pool.tile([128, 512], fp32, tag="mm",
                                   name=f"mm{tg}_0_{j}")
                    for j in range(4)
                ]
                for k in range(KC):
                    nc.tensor.matmul(
                        up[:], aext_tiles[k][:], xs_tiles[k][:],
                        start=(k == 0), stop=(k == KC - 1),
                    )
                    lhsT = xs_tiles[k][:, 0:128]
                    for j in range(4):
                        nc.tensor.matmul(
                            ps0[j][:], lhsT,
                            nib_tiles[k][:, j * 512:(j + 1) * 512],
                            start=(k == 0), stop=False,
                        )
                gt = g_pool.tile([9, TG * 128], fp16, tag="g")
                nc.vector.tensor_copy(gt[:, :], up[:])

                for tt in range(TG):
                    if tt == 0:
                        ps = ps0
                    else:
                        ps = [
                            psum_pool.tile([128, 512], fp32, tag="mm",
                                           name=f"mm{tg}_{tt}_{j}")
                            for j in range(4)
                        ]
                        for k in range(KC):
                            lhsT = xs_tiles[k][:, tt * 128:(tt + 1) * 128]
                            for j in range(4):
                                nc.tensor.matmul(
                                    ps[j][:], lhsT,
                                    nib_tiles[k][:, j * 512:(j + 1) * 512],
                                    start=(k == 0), stop=False,
                                )
                    gs = gt[:, tt * 128:(tt + 1) * 128]
                    for j in range(4):
                        nc.tensor.matmul(
                            ps[j][:], gs, h_tile[:, j * 512:(j + 1) * 512],
                            start=False, stop=True,
                        )
                    ot = out_pool.tile([128, OC], fp32, tag="out")
                    for j in range(4):
                        nc.vector.tensor_add(
                            ot[:, j * 512:(j + 1) * 512], ps[j][:],
                            bias_tile[:, j * 512:(j + 1) * 512],
                        )
                    trow = t0 + tt * 128
                    nc.sync.dma_start(y[trow:trow + 128, :], ot[:])
    nc.compile()
    return nc


def _prep_inputs(x, weight_quant, scale, zero, lora_A, lora_B, bias):
    """Host-side layout prep + sharding. Returns in_maps for 8 cores."""
    xs = (x.reshape(T, I).astype(np.float32) * scale[None, :]).astype(np.float16)
    xsT = np.ascontiguousarray(xs.T)  # [I, T]

    wq = weight_quant.astype(np.uint8)            # low byte only is populated
    nib = np.empty((O, I), np.uint8)
    nib[:, 0::2] = wq & 15
    nib[:, 1::2] = wq >> 4
    import ml_dtypes
    nibT = np.ascontiguousarray(nib.T.astype(ml_dtypes.float8_e4m3fn))  # [I, O]

    aextT = np.empty((I, 9), np.float16)
    aextT[:, 0:8] = (lora_A.astype(np.float32) / scale[None, :]).T
    aextT[:, 8] = zero
    aextT = np.ascontiguousarray(aextT)

    hmat = np.empty((9, O), np.float16)
    hmat[0:8, :] = 2.0 * lora_B.T
    hmat[8, :] = -1.0
    hmat = np.ascontiguousarray(hmat)
    bias_bc = np.broadcast_to(bias.astype(np.float32), (128, O))

    in_maps = []
    for c in range(8):
        q, h = divmod(c, NH)
        in_maps.append({
            "xsT": np.ascontiguousarray(xsT[:, q * TC:(q + 1) * TC]),
            "nibT": np.ascontiguousarray(nibT[:, h * OC:(h + 1) * OC]),
            "aextT": aextT,
            "hmat": np.ascontiguousarray(hmat[:, h * OC:(h + 1) * OC]),
            "bias_bc": np.ascontiguousarray(bias_bc[:, h * OC:(h + 1) * OC]),
        })
    return in_maps


def run_on_cores(in_maps, trace=False):
    from concourse.bass_utils import run_bass_kernel_spmd

    if "nc" not in _CACHE:
        _CACHE["nc"] = _build_program()
    return run_bass_kernel_spmd(
        _CACHE["nc"], in_maps, list(range(8)), trace=trace
    )


def kernel(x, weight_quant, scale, zero, lora_A, lora_B, bias):
    x = np.asarray(x)
    weight_quant = np.asarray(weight_quant)
    scale = np.asarray(scale, np.float32)
    zero = np.asarray(zero, np.float32)
    lora_A = np.asarray(lora_A, np.float32)
    lora_B = np.asarray(lora_B, np.float32)
    bias = np.asarray(bias, np.float32)

    in_maps = _prep_inputs(x, weight_quant, scale, zero, lora_A, lora_B, bias)
    res = run_on_cores(in_maps).results

    out = np.empty((T, O), np.float32)
    for c in range(8):
        q, h = divmod(c, NH)
        out[q * TC:(q + 1) * TC, h * OC:(h + 1) * OC] = res[c]["y"]
    return out.reshape(B, S, O)
